# revision 18
# baseline (speedup 1.0000x reference)
"""Trainium2 Bass kernel for nn_CorrClassLoss.

Reference computation (B=4, C=19, H=512, W=1024, N=5000, IGNORE=255):
  ref_class = argmax_c inputs_ref[b].reshape(C, H*W)      # flat W-major
  lin_ref   = 512*y_ref + x_ref    (NOTE: linearized with H, kept faithfully)
  lin_other = 512*y_other + x_other
  gathered  = ref_class[b, lin_ref]
  target[b, lin_other] = gathered  (scatter, last write wins; rest IGNORE)
  loss = mean over non-ignored pixels of -log_softmax(inputs_other)[b, target, px]

Since lin = 512*y + x with x,y in [0,512), only flat positions [0, 262144)
are ever touched, and at most N unique scatter destinations per batch
contribute to the loss:

  loss = -(1/cnt) * sum over unique dests d (last writer j, src s_j) of
         [ x_other[b, cls(s_j), d] - ln(sum_c exp(x_other[b, c, d])) ]
  cls(s) = argmax_c x_ref[b, c, s],  cnt = total unique dests.

Strategy (8 cores, data-parallel over (batch, half-of-correspondences)):
  Host does index-only math (dedup last-wins, split j by the pixel-half of
  s_j, pack a padded row-offset table) and hands each core a pixel-major
  transposed shard cat_t = [ref_half | other_full | zero-row] (a layout/
  sharding choice; all value compute happens on device).

  Device per core: SWDGE indirect gathers (the HW consumes one dynamic
  offset per partition per instruction, so a [128]-row column per
  instruction is the ISA-maximal batch).  Row-unit offsets against the 2D
  table keep each descriptor at one 19-float class vector.  Padded slots
  point at the trailing all-zero row so every element is written
  deterministically.  Gathers stream back-to-back on the Pool engine;
  compute is chunked and pipelined in their shadow:
    term1: grouped max -> is_ge one-hot -> mult by other -> XY-reduce
           into one accumulator column per chunk                 (DVE)
    term2: exp (Act) -> grouped sum (DVE) -> one final Ln with fused
           per-partition accumulate over all groups              (Act)
  A pre-placed LoadActFuncSet pins the combined exp+ln table so no
  1.28us table switch lands mid-stream.
  Pad slots contribute 0 to term1 and ln(19) each to term2 (they gather
  the zero row), which the host corrects with a closed-form constant.
  Output [128, CH+1] per-partition partial sums; host does the final
  (sum, count) combine across cores per the data-parallel recipe.
"""

import sys

if "/opt/trn_rl_repo" not in sys.path:
    sys.path.insert(0, "/opt/trn_rl_repo")

import numpy as np

B, C, H, W = 4, 19, 512, 1024
HW = H * W                 # 524288
NPIX = 262144              # touched flat range [0, 262144)
NPIX_H = NPIX // 2         # 131072 source pixels per core
N = 5000
NCORES = 8

P = 128                    # partitions
ZROW = NPIX_H + NPIX       # index of the all-zero row in cat_t
CHUNK = 4                  # gather columns per compute chunk

_programs = {}


def _build_program(cgg):
    import concourse.bass as bass
    import concourse.bacc as bacc
    import concourse.mybir as mybir
    import concourse.tile as tile

    # chunk sizes: CHUNK-wide, but keep the final chunk 1 column so the
    # tail compute after the very last gather is minimal
    sizes = _chunk_sizes(cgg)
    nch = len(sizes)

    nc = bacc.Bacc("TRN2", target_bir_lowering=False, debug=False,
                   num_devices=NCORES)

    # [ref_half | other_full | zero-row], pixel-major transposed
    cat_t = nc.dram_tensor("cat_t", [ZROW + 1, C], mybir.dt.float32,
                           kind="ExternalInput")
    # row offsets in chunk order: for each chunk [others(w) | refs(w)];
    # pads point at ZROW
    off = nc.dram_tensor("off", [P, 2 * cgg], mybir.dt.int32,
                         kind="ExternalInput")
    out = nc.dram_tensor("out", [P, nch + 1], mybir.dt.float32,
                         kind="ExternalOutput")

    with tile.TileContext(nc) as tc:
        with tc.tile_pool(name="gb", bufs=1) as gb:
            so = gb.tile([P, 2 * cgg], mybir.dt.int32)
            nc.sync.dma_start(out=so[:], in_=off[:, :])
            # pre-place the combined exp+ln activation table load so the
            # table-load pass doesn't inject a 1.28us switch mid-stream
            nc.scalar.add_instruction(
                mybir.InstLoadActFuncSet(
                    name=nc.scalar.bass.get_next_instruction_name(),
                    act_func_set_id=6,   # natural_log_exp_and_others
                )
            )

            G = gb.tile([P, cgg * 19], mybir.dt.float32)     # ref vectors
            G2 = gb.tile([P, cgg * 19], mybir.dt.float32)    # other vectors
            TP = gb.tile([P, nch + 1], mybir.dt.float32)
            m2 = gb.tile([P, cgg], mybir.dt.float32)
            eq = gb.tile([P, cgg * 19], mybir.dt.float32)
            sc = gb.tile([P, cgg * 19], mybir.dt.float32)
            e2 = gb.tile([P, cgg * 19], mybir.dt.float32)
            S2 = gb.tile([P, cgg], mybir.dt.float32)

            def gather(dst, col, off_col):
                nc.gpsimd.indirect_dma_start(
                    out=dst[:, col * 19:(col + 1) * 19],
                    out_offset=None,
                    in_=cat_t[:, :],
                    in_offset=bass.IndirectOffsetOnAxis(
                        ap=so[:, off_col:off_col + 1], axis=0),
                )

            lo = 0
            oc = 0                 # running offset-column pointer
            for k, w in enumerate(sizes):
                hi = lo + w
                # other gathers first: the Act exp/S2 chain starts while
                # the ref gathers of this chunk are still generating
                for i, col in enumerate(range(lo, hi)):
                    gather(G2, col, oc + i)
                for i, col in enumerate(range(lo, hi)):
                    gather(G, col, oc + w + i)
                oc += 2 * w

                sl = slice(lo * 19, hi * 19)
                slg = slice(lo, hi)
                R2c = G2[:, sl]
                nc.scalar.activation(e2[:, sl], R2c,
                                     mybir.ActivationFunctionType.Exp)
                nc.vector.tensor_reduce(
                    out=S2[:, slg],
                    in_=e2[:, sl].rearrange("p (g c) -> p g c", c=19),
                    axis=mybir.AxisListType.X,
                    op=mybir.AluOpType.add,
                )
                Rc = G[:, sl].rearrange("p (g c) -> p g c", c=19)
                nc.vector.tensor_reduce(out=m2[:, slg], in_=Rc,
                                        axis=mybir.AxisListType.X,
                                        op=mybir.AluOpType.max)
                nc.vector.tensor_tensor(
                    out=eq[:, sl].rearrange("p (g c) -> p g c", c=19),
                    in0=Rc,
                    in1=m2[:, slg, None].to_broadcast([P, w, 19]),
                    op=mybir.AluOpType.is_ge,
                )
                nc.vector.tensor_tensor(
                    out=sc[:, sl], in0=eq[:, sl], in1=R2c,
                    op=mybir.AluOpType.mult,
                )
                nc.vector.tensor_reduce(
                    out=TP[:, k:k + 1],
                    in_=sc[:, sl].rearrange("p (g c) -> p g c", c=19),
                    axis=mybir.AxisListType.XY,
                    op=mybir.AluOpType.add,
                )
                lo = hi

            # term2 tail: one Ln over all groups, fused accumulate
            L2 = gb.tile([P, cgg], mybir.dt.float32)
            nc.scalar.activation(L2[:], S2[:],
                                 mybir.ActivationFunctionType.Ln,
                                 accum_out=TP[:, nch:nch + 1])

            nc.sync.dma_start(out=out[:, :], in_=TP[:])

    nc.finalize()
    return nc


def _get_program(cgg):
    if cgg not in _programs:
        _programs[cgg] = _build_program(cgg)
    return _programs[cgg]


def _host_prep(inds_ref, inds_other):
    """Index-only host math: dedup scatter (last wins), partition per core."""
    ir = np.asarray(inds_ref).astype(np.int64)      # [B, 2, N]
    io = np.asarray(inds_other).astype(np.int64)
    valid = ((ir[:, 0] >= 0) & (ir[:, 0] < W) & (ir[:, 1] >= 0) & (ir[:, 1] < H)
             & (io[:, 0] >= 0) & (io[:, 0] < W) & (io[:, 1] >= 0)
             & (io[:, 1] < H))                       # [B, N]
    lin_ref = H * ir[:, 1] + ir[:, 0]                # [B, N]
    lin_other = H * io[:, 1] + io[:, 0]

    per_core = []
    count = 0
    for b in range(B):
        v = valid[b]
        lo = lin_other[b][v]
        lr = np.clip(lin_ref[b][v], 0, HW - 1)
        # last-write-wins dedup on destinations
        u, first_rev = np.unique(lo[::-1], return_index=True)
        last_idx = len(lo) - 1 - first_rev
        d_arr = u.astype(np.int64)
        s_arr = lr[last_idx].astype(np.int64)
        count += len(u)
        for h in range(2):
            sel = (s_arr // NPIX_H) == h
            per_core.append({
                "b": b, "h": h,
                "s": s_arr[sel] - h * NPIX_H,
                "d": d_arr[sel],
            })
    return per_core, count


def _chunk_sizes(cgg):
    if cgg == 1:
        return [1]
    body = cgg - 1
    sizes = [CHUNK] * (body // CHUNK)
    if body % CHUNK:
        sizes.append(body % CHUNK)
    sizes.append(1)
    return sizes


def _pack_off(pc, cgg):
    """Row-offset table [P, 2*cgg] in chunk order (others|refs per chunk);
    pads target the zero row."""
    o_ref = np.full((P, cgg), ZROW, dtype=np.int32)
    o_oth = np.full((P, cgg), ZROW, dtype=np.int32)
    s, d = pc["s"], pc["d"]
    n = len(s)
    assert n <= cgg * P
    jj = np.arange(n)
    o_ref[jj % P, jj // P] = s
    o_oth[jj % P, jj // P] = NPIX_H + d
    cols = []
    lo = 0
    for w in _chunk_sizes(cgg):
        cols.append(o_oth[:, lo:lo + w])
        cols.append(o_ref[:, lo:lo + w])
        lo += w
    return np.concatenate(cols, axis=1)


def _make_in_maps(inputs_ref, inputs_other, per_core, cgg):
    ref_flat = inputs_ref.reshape(B, C, HW)
    other_flat = inputs_other.reshape(B, C, HW)
    in_maps = []
    for pc in per_core:
        b, h = pc["b"], pc["h"]
        cat = np.empty((ZROW + 1, C), dtype=np.float32)
        cat[:NPIX_H] = ref_flat[b, :, h * NPIX_H:(h + 1) * NPIX_H].T
        cat[NPIX_H:ZROW] = other_flat[b, :, :NPIX].T
        cat[ZROW] = 0.0
        in_maps.append({
            "cat_t": cat,
            "off": _pack_off(pc, cgg),
        })
    return in_maps


def kernel(inputs_ref, inputs_other, inds_ref, inds_other, weights):
    from concourse.bass_utils import run_bass_kernel_spmd

    inputs_ref = np.asarray(inputs_ref, dtype=np.float32)
    inputs_other = np.asarray(inputs_other, dtype=np.float32)

    per_core, count = _host_prep(inds_ref, inds_other)
    # exact-fit capacity: compile (and cache) for the worst-core
    # correspondence count, rounded up to whole 128-columns
    max_n = max(len(pc["s"]) for pc in per_core)
    cgg = max(1, -(-max_n // P))
    nc = _get_program(cgg)

    in_maps = _make_in_maps(inputs_ref, inputs_other, per_core, cgg)
    res = run_bass_kernel_spmd(nc, in_maps, core_ids=list(range(NCORES)))
    total = 0.0
    npad_total = 0
    for pc, r in zip(per_core, res.results):
        o = np.asarray(r["out"], dtype=np.float64)
        total += o[:, :-1].sum() - o[:, -1].sum()
        npad_total += cgg * P - len(pc["s"])
    # pad slots gather the zero row: term1 += 0, term2 += ln(19) each
    total += np.log(19.0) * npad_total
    loss = -total / max(count, 1)
    return np.float32(loss)
